# revision 24
# baseline (speedup 1.0000x reference)
"""Trainium2 Bass kernel for nn_RecurrentSpatialQNetwork.

Strategy (pure data parallel over 8 NeuronCores, batch 65536 -> 8192/core):
- Feature-major on-chip layout: activations stored [features(partitions), batch(free)],
  batch processed in 16 tiles of 512 columns per core.
- conv1 + pos/met/aff encoders fused into one [50 x 496] Toeplitz matmul.
- conv2 as dense [400 x 800] Toeplitz matmul.
- All matmuls in float32r (full-rate fp32 on the PE at N=512).
- LayerNorm stats via ones-matrix matmuls that produce mean/meansq already
  broadcast across partitions ([128,512] PSUM tiles).
- h0 = c0 = 0 exploited (spec fill=zeros): w_hh matmul and the forget gate are
  dropped; LN_h is folded into the q1 matmul. A numpy fallback handles the
  general (nonzero h0/c0) case exactly.
- Three phases per core (encoder / LSTM / Q-head) so the ScalarEngine only
  switches activation-table sets twice (sqrt set -> sigmoid set -> sqrt set).
"""
import os
import sys

for _p in ("/opt/trn_rl_repo", "/root/.axon_site/_ro/trn_rl_repo"):
    if os.path.isdir(_p) and _p not in sys.path:
        sys.path.append(_p)

import numpy as np

B = 65536
WIN, POS, MET, AFF, HID, NACT = 5, 2, 8, 15, 256, 8
N_CORES = 8
B_CORE = B // N_CORES  # 8192
TILE = 512
N_TILES = B_CORE // TILE  # 16
EPS = 1e-5

_CACHE = {}


# --------------------------------------------------------------------------
# host-side weight preparation
# --------------------------------------------------------------------------
def _prep_weights(p):
    f32 = np.float32
    conv1_w, conv2_w = np.asarray(p["conv1_w"]), np.asarray(p["conv2_w"])
    W0 = np.zeros((50, 496), f32)
    for co in range(16):
        for r in range(WIN):
            for c in range(WIN):
                o = co * 25 + r * 5 + c
                for dr in (-1, 0, 1):
                    for dc in (-1, 0, 1):
                        rr, cc = r + dr, c + dc
                        if 0 <= rr < 5 and 0 <= cc < 5:
                            W0[rr * 5 + cc, o] = conv1_w[co, 0, dr + 1, dc + 1]
    W0[25:27, 400:432] = p["pos_w"].T
    W0[27:35, 432:464] = p["met_w"].T
    W0[35:50, 464:496] = p["aff_w"].T
    b0 = np.concatenate(
        [np.repeat(p["conv1_b"], 25), p["pos_b"], p["met_b"], p["aff_b"]]
    ).astype(f32)

    K2 = np.zeros((400, 800), f32)
    for co in range(32):
        for ci in range(16):
            for r in range(WIN):
                for c in range(WIN):
                    o = co * 25 + r * 5 + c
                    for dr in (-1, 0, 1):
                        for dc in (-1, 0, 1):
                            rr, cc = r + dr, c + dc
                            if 0 <= rr < 5 and 0 <= cc < 5:
                                K2[ci * 25 + rr * 5 + cc, o] = conv2_w[
                                    co, ci, dr + 1, dc + 1
                                ]
    b2 = np.repeat(p["conv2_b"], 25).astype(f32)

    igo = np.r_[0:256, 512:1024]
    W2 = p["q1_w"] * p["ln_g"][None, :]
    return dict(
        w0=np.ascontiguousarray(W0),
        k2=np.ascontiguousarray(K2),
        vist=np.ascontiguousarray(p["vis_w"].T.astype(f32)),
        wiht=np.ascontiguousarray(p["w_ih"][igo, :].T.astype(f32)),
        w2t=np.ascontiguousarray(W2.T.astype(f32)),
        q2t=np.ascontiguousarray(p["q2_w"].T.astype(f32)),
        b0=b0.reshape(-1, 1),
        b2=b2.reshape(-1, 1),
        bg=(p["b_ih"] + p["b_hh"])[igo].astype(f32).reshape(-1, 1),
        w0col=(p["q1_w"] @ p["ln_b"] + p["q1_b"]).astype(f32).reshape(-1, 1),
        negv=(-W2.sum(axis=1)).astype(f32).reshape(-1, 1),
        vis_g=np.asarray(p["vis_g"], f32).reshape(-1, 1),
        vis_beta=np.asarray(p["vis_beta"], f32).reshape(-1, 1),
        q1_g=np.asarray(p["q1_g"], f32).reshape(-1, 1),
        q1_beta=np.asarray(p["q1_beta"], f32).reshape(-1, 1),
        q2_b=np.asarray(p["q2_b"], f32).reshape(-1, 1),
        ones128=np.full((128, 128), 1.0 / 128.0, f32),
        ones256=np.full((128, 128), 1.0 / 256.0, f32),
        negones128=np.full((128, 128), -1.0 / 128.0, f32),
    )


WEIGHT_SHAPES = {
    "w0": [50, 496],
    "k2": [400, 800],
    "vist": [800, 128],
    "wiht": [224, 768],
    "w2t": [256, 128],
    "q2t": [128, 8],
    "b0": [496, 1],
    "b2": [800, 1],
    "bg": [768, 1],
    "w0col": [128, 1],
    "negv": [128, 1],
    "vis_g": [128, 1],
    "vis_beta": [128, 1],
    "q1_g": [128, 1],
    "q1_beta": [128, 1],
    "q2_b": [8, 1],
    "ones128": [128, 128],
    "ones256": [128, 128],
    "negones128": [128, 128],
}


# --------------------------------------------------------------------------
# walrus workaround: this container's walrus accepts only ONE sync wait per
# instruction; split extras into preceding same-engine NoOps.
# --------------------------------------------------------------------------
def _split_multi_waits(nc, mybir):
    n = 0
    for f in nc.m.functions:
        for blk in f.blocks:
            out = []
            changed = False
            for inst in blk.instructions:
                si = inst.sync_info
                if si is not None and len(si.on_wait) > 1:
                    waits = list(si.on_wait)
                    for j, w in enumerate(waits[:-1]):
                        nop = mybir.InstNoOp(name=f"{inst.name}-wsplit{j}")
                        nop.engine = inst.engine
                        nop.sync_info = mybir.SyncInfo(on_wait=[w], on_update=[])
                        out.append(nop)
                        n += 1
                    inst.sync_info = mybir.SyncInfo(
                        on_wait=[waits[-1]], on_update=list(si.on_update)
                    )
                    changed = True
                out.append(inst)
            if changed:
                blk.instructions = out
    return n


# --------------------------------------------------------------------------
# bass kernel construction
# --------------------------------------------------------------------------
def _build_nc():
    import concourse.bass as bass
    import concourse.tile as tile
    from concourse import mybir

    dt = mybir.dt
    F32, F16 = dt.float32, dt.float16
    AF = mybir.ActivationFunctionType
    OP = mybir.AluOpType
    CH = 2 * TILE  # 1024-wide chunks, two 512 matmul halves
    N_CHUNKS = B_CORE // CH  # 8

    nc = bass.Bass()

    def act_rsqrt(out, in_, bias_ap):
        eng = nc.scalar
        ins = [
            eng.lower_ap(in_),
            eng.lower_ap(bias_ap),
            mybir.ImmediateValue(dtype=F32, value=1.0),
            mybir.ImmediateValue(dtype=F32, value=0.0),
        ]
        return eng.add_instruction(
            mybir.InstActivation(
                name=nc.get_next_instruction_name(),
                func=AF.Rsqrt,
                ins=ins,
                outs=[eng.lower_ap(out)],
            )
        )

    MM_P = {"w0", "k2", "vist", "wiht", "w2t", "q2t", "ones128", "ones256",
            "negones128"}
    obs_t = nc.declare_dram_parameter("obs_t", [50, B_CORE], F16, isOutput=False)
    wd = {
        k: nc.declare_dram_parameter(k, shp, F16 if k in MM_P else F32, isOutput=False)
        for k, shp in WEIGHT_SHAPES.items()
    }
    q_t = nc.declare_dram_parameter("q_t", [NACT, B_CORE], F32, isOutput=True)
    h_t = nc.declare_dram_parameter("h_t", [HID, B_CORE], F32, isOutput=True)
    c_t = nc.declare_dram_parameter("c_t", [HID, B_CORE], F32, isOutput=True)

    with tile.TileContext(nc) as tc:
        ctx_pools = []

        def pool(name, bufs, space="SBUF"):
            p = tc.tile_pool(name=name, bufs=bufs, space=space)
            ctx_pools.append(p)
            return p.__enter__()

        singles = pool("singles", 1)
        w0_sb = singles.tile([50, 496], F16, tag="w0")
        nc.sync.dma_start(w0_sb[:], wd["w0"][:])
        k2_sb = []
        for j, kn in enumerate((128, 128, 128, 16)):
            t = singles.tile([kn, 800], F16, tag=f"k2_{j}")
            nc.sync.dma_start(t[:], wd["k2"][j * 128 : j * 128 + kn, :])
            k2_sb.append(t)
        vis_sb = []
        for j in range(7):
            kn = 128 if j < 6 else 32
            t = singles.tile([kn, 128], F16, tag=f"vis_{j}")
            nc.sync.dma_start(t[:], wd["vist"][j * 128 : j * 128 + kn, :])
            vis_sb.append(t)
        wih_sb = []
        for j, kn in enumerate((128, 96)):
            t = singles.tile([kn, 768], F16, tag=f"wih_{j}")
            nc.sync.dma_start(t[:], wd["wiht"][j * 128 : j * 128 + kn, :])
            wih_sb.append(t)
        w2t_sb = []
        for j in range(2):
            t = singles.tile([128, 128], F16, tag=f"w2t_{j}")
            nc.sync.dma_start(t[:], wd["w2t"][j * 128 : (j + 1) * 128, :])
            w2t_sb.append(t)
        q2t_sb = singles.tile([128, NACT], F16, tag="q2t")
        nc.sync.dma_start(q2t_sb[:], wd["q2t"][:])
        ones128 = singles.tile([128, 128], F16, tag="ones128")
        nc.sync.dma_start(ones128[:], wd["ones128"][:])
        ones256 = singles.tile([128, 128], F16, tag="ones256")
        nc.sync.dma_start(ones256[:], wd["ones256"][:])
        negones128 = singles.tile([128, 128], F16, tag="negones128")
        nc.sync.dma_start(negones128[:], wd["negones128"][:])

        b0_sb = []
        for j, (p0, pn) in enumerate(((0, 128), (128, 128), (256, 128), (384, 16), (400, 96))):
            t = singles.tile([pn, 1], F32, tag=f"b0_{j}")
            nc.sync.dma_start(t[:], wd["b0"][p0 : p0 + pn, :])
            b0_sb.append(t)
        b2_sb = []
        for j in range(7):
            pn = 128 if j < 6 else 32
            t = singles.tile([pn, 1], F32, tag=f"b2_{j}")
            nc.sync.dma_start(t[:], wd["b2"][j * 128 : j * 128 + pn, :])
            b2_sb.append(t)
        bg_sb = []
        for j in range(6):
            t = singles.tile([128, 1], F32, tag=f"bg_{j}")
            nc.sync.dma_start(t[:], wd["bg"][j * 128 : (j + 1) * 128, :])
            bg_sb.append(t)
        small_vecs = {}
        for name, pn in (
            ("negv", 128),
            ("w0col", 128),
            ("vis_g", 128),
            ("vis_beta", 128),
            ("q1_g", 128),
            ("q1_beta", 128),
            ("q2_b", 8),
        ):
            t = singles.tile([pn, 1], F32, tag=f"sv_{name}")
            nc.sync.dma_start(t[:], wd[name][:])
            small_vecs[name] = t
        eps_sb = singles.tile([128, 1], F32, tag="epsvec")
        nc.vector.memset(eps_sb[:], EPS)

        # ---- pools
        xpool = pool("x0", 3)
        a1pool = pool("a1", 5)
        zpool = pool("z", 7)
        tmpA = pool("tmpA", 9)
        sqpool = pool("sq", 4)
        xspool = pool("xs", 3)
        stspool = pool("sts", 8)
        vopool = pool("vo", 4)       # vision tiles passed A->B
        e96pool = pool("e96", 4)     # enc96 tiles passed A->B
        gpool = pool("gates", 4)
        chpool = pool("ch", 3)       # c + tanh_c f32 temps
        hpool = pool("hh", 6)        # h fp16 tiles passed B->C
        tmpCr = pool("tmpCr", 6)
        qpool = pool("qout", 2)
        psbig = pool("psbig", 3, space="PSUM")
        psstat = pool("psstat", 2, space="PSUM")

        HALVES = (slice(0, TILE), slice(TILE, CH))

        vis_tiles = {}
        enc_tiles = {}
        h_tiles = {}

        # ---------------- phase emitters ----------------
        def emit_A(t):
            cols = slice(t * CH, (t + 1) * CH)
            x0 = xpool.tile([50, CH], F16, tag="x0")
            nc.sync.dma_start(x0[:], obs_t[:, cols])

            a1 = []
            enc96 = e96pool.tile([96, CH], F16, tag="enc96")
            for j, (m0, mn) in enumerate(
                ((0, 128), (128, 128), (256, 128), (384, 16), (400, 96))
            ):
                ps = psbig.tile([mn, CH], F32, tag="ps")
                for h, hs_ in enumerate(HALVES):
                    nc.tensor.matmul(
                        ps[:, hs_], w0_sb[:, m0 : m0 + mn], x0[:, hs_],
                        start=True, stop=True,
                    )
                if j < 3:
                    dst = a1pool.tile([128, CH], F16, tag="a1")
                    if j % 2 == 0:
                        nc.scalar.activation(dst[:], ps[:], AF.Relu, bias=b0_sb[j][:])
                    else:
                        nc.vector.tensor_scalar(
                            dst[:], ps[:], b0_sb[j][:], 0.0, OP.add, OP.max
                        )
                    a1.append(dst)
                elif j == 3:
                    g16 = xpool.tile([16, CH], F16, tag="g16")
                    nc.vector.tensor_scalar(
                        g16[:], ps[:], b0_sb[3][:], 0.0, OP.add, OP.max
                    )
                    a1.append(g16)
                else:
                    nc.scalar.activation(enc96[:], ps[:], AF.Relu, bias=b0_sb[4][:])
            enc_tiles[t] = enc96

            z = []
            for j in range(7):
                m0, mn = j * 128, (128 if j < 6 else 32)
                ps = psbig.tile([mn, CH], F32, tag="ps")
                for h, hs_ in enumerate(HALVES):
                    for kj in range(4):
                        nc.tensor.matmul(
                            ps[:, hs_], k2_sb[kj][:, m0 : m0 + mn], a1[kj][:, hs_],
                            start=(kj == 0), stop=(kj == 3),
                        )
                dst = zpool.tile([mn, CH], F16, tag="z")
                if j % 2 == 0:
                    nc.vector.tensor_scalar(
                        dst[:], ps[:], b2_sb[j][:], 0.0, OP.add, OP.max
                    )
                else:
                    nc.scalar.activation(dst[:], ps[:], AF.Relu, bias=b2_sb[j][:])
                z.append(dst)

            psv = psbig.tile([128, CH], F32, tag="ps")
            for h, hs_ in enumerate(HALVES):
                for kj in range(7):
                    nc.tensor.matmul(
                        psv[:, hs_], vis_sb[kj][:], z[kj][:, hs_],
                        start=(kj == 0), stop=False,
                    )
            # center-in-PSUM LN
            x_s = xspool.tile([128, CH], F16, tag="xs")
            nc.vector.tensor_copy(x_s[:], psv[:])
            for h, hs_ in enumerate(HALVES):
                nc.tensor.matmul(
                    psv[:, hs_], negones128[:], x_s[:, hs_], start=False, stop=True
                )
            xcsq = sqpool.tile([128, CH], F16, tag="sq")
            nc.scalar.activation(xcsq[:], psv[:], AF.Square)
            rr = tmpA.tile([128, CH], F32, tag="tA")
            for h, hs_ in enumerate(HALVES):
                qb = psstat.tile([128, TILE], F32, tag="st")
                nc.tensor.matmul(qb[:], ones128[:], xcsq[:, hs_], start=True, stop=True)
                act_rsqrt(rr[:, hs_], qb[:], eps_sb[:])
            e = tmpA.tile([128, CH], F32, tag="tA")
            nc.vector.tensor_tensor(e[:], psv[:], rr[:], OP.mult)
            vis = vopool.tile([128, CH], F16, tag="visout")
            nc.scalar.activation(
                vis[:], e[:], AF.Relu,
                bias=small_vecs["vis_beta"][:], scale=small_vecs["vis_g"][:],
            )
            vis_tiles[t] = vis

        GATE_FN = (AF.Sigmoid, AF.Sigmoid, AF.Tanh, AF.Tanh, AF.Sigmoid, AF.Sigmoid)

        def emit_B(t):
            cols = slice(t * CH, (t + 1) * CH)
            vis = vis_tiles.pop(t)
            enc96 = enc_tiles.pop(t)
            gates = []
            for mj in range(6):
                ps = psbig.tile([128, CH], F32, tag="ps")
                for h, hs_ in enumerate(HALVES):
                    nc.tensor.matmul(
                        ps[:, hs_], wih_sb[0][:, mj * 128 : (mj + 1) * 128],
                        vis[:, hs_], start=True, stop=False,
                    )
                    nc.tensor.matmul(
                        ps[:, hs_], wih_sb[1][:, mj * 128 : (mj + 1) * 128],
                        enc96[:, hs_], start=False, stop=True,
                    )
                gt = gpool.tile([128, CH], F32, tag="gate")
                nc.scalar.activation(gt[:], ps[:], GATE_FN[mj], bias=bg_sb[mj][:])
                gates.append(gt)
            hts = []
            for j in range(2):
                cj = chpool.tile([128, CH], F32, tag="ch")
                nc.vector.tensor_tensor(cj[:], gates[j][:], gates[2 + j][:], OP.mult)
                nc.sync.dma_start(c_t[j * 128 : (j + 1) * 128, cols], cj[:])
                tcj = chpool.tile([128, CH], F32, tag="ch")
                nc.scalar.activation(tcj[:], cj[:], AF.Tanh)
                hj = chpool.tile([128, CH], F32, tag="ch32")
                nc.vector.tensor_tensor(hj[:], gates[4 + j][:], tcj[:], OP.mult)
                nc.sync.dma_start(h_t[j * 128 : (j + 1) * 128, cols], hj[:])
                h16 = hpool.tile([128, CH], F16, tag="hh")
                nc.vector.tensor_copy(h16[:], hj[:])
                hts.append(h16)
            h_tiles[t] = hts

        def emit_C(t):
            cols = slice(t * CH, (t + 1) * CH)
            hs = h_tiles.pop(t)
            hsq = []
            for j in range(2):
                s = sqpool.tile([128, CH], F16, tag="sq")
                nc.scalar.activation(s[:], hs[j][:], AF.Square)
                hsq.append(s)
            r2 = tmpA.tile([128, CH], F32, tag="tA")
            mcp2 = []
            for h, hs_ in enumerate(HALVES):
                mb = psstat.tile([128, TILE], F32, tag="st")
                for kb in range(2):
                    nc.tensor.matmul(
                        mb[:], ones256[:], hs[kb][:, hs_],
                        start=(kb == 0), stop=(kb == 1),
                    )
                mcp = stspool.tile([128, TILE], F32, tag="sts")
                nc.vector.tensor_copy(mcp[:], mb[:])
                mcp2.append(mcp)
                qb = psstat.tile([128, TILE], F32, tag="st")
                for kb in range(2):
                    nc.tensor.matmul(
                        qb[:], ones256[:], hsq[kb][:, hs_],
                        start=(kb == 0), stop=(kb == 1),
                    )
                msq = stspool.tile([128, TILE], F32, tag="sts")
                nc.scalar.activation(msq[:], mcp[:], AF.Square)
                rin = stspool.tile([128, TILE], F32, tag="sts")
                nc.vector.scalar_tensor_tensor(
                    rin[:], qb[:], 1.0, msq[:], OP.mult, OP.subtract
                )
                act_rsqrt(r2[:, hs_], rin[:], eps_sb[:])
            psu = psbig.tile([128, CH], F32, tag="ps")
            for h, hs_ in enumerate(HALVES):
                nc.tensor.matmul(
                    psu[:, hs_], w2t_sb[0][:], hs[0][:, hs_], start=True, stop=False
                )
                nc.tensor.matmul(
                    psu[:, hs_], w2t_sb[1][:], hs[1][:, hs_], start=False, stop=True
                )
            y1 = tmpA.tile([128, CH], F32, tag="tA")
            mr = tmpA.tile([128, CH], F32, tag="tA")
            for h, hs_ in enumerate(HALVES):
                nc.vector.scalar_tensor_tensor(
                    y1[:, hs_], psu[:, hs_], small_vecs["w0col"][:], r2[:, hs_],
                    OP.add, OP.mult,
                )
                nc.vector.tensor_tensor(mr[:, hs_], mcp2[h][:], r2[:, hs_], OP.mult)
            y = tmpCr.tile([128, CH], F16, tag="tC_r")
            nc.vector.scalar_tensor_tensor(
                y[:], mr[:], small_vecs["negv"][:], y1[:], OP.mult, OP.add
            )
            ysq = sqpool.tile([128, CH], F16, tag="sq")
            nc.scalar.activation(ysq[:], y[:], AF.Square)
            r3 = tmpA.tile([128, CH], F32, tag="tA")
            mcp3 = []
            for h, hs_ in enumerate(HALVES):
                mb = psstat.tile([128, TILE], F32, tag="st")
                nc.tensor.matmul(mb[:], ones128[:], y[:, hs_], start=True, stop=True)
                mcp = stspool.tile([128, TILE], F32, tag="sts")
                nc.vector.tensor_copy(mcp[:], mb[:])
                mcp3.append(mcp)
                qb = psstat.tile([128, TILE], F32, tag="st")
                nc.tensor.matmul(qb[:], ones128[:], ysq[:, hs_], start=True, stop=True)
                msq = stspool.tile([128, TILE], F32, tag="sts")
                nc.scalar.activation(msq[:], mcp[:], AF.Square)
                rin = stspool.tile([128, TILE], F32, tag="sts")
                nc.vector.scalar_tensor_tensor(
                    rin[:], qb[:], 1.0, msq[:], OP.mult, OP.subtract
                )
                act_rsqrt(r3[:, hs_], rin[:], eps_sb[:])
            d3 = tmpA.tile([128, CH], F32, tag="tA")
            for h, hs_ in enumerate(HALVES):
                nc.vector.scalar_tensor_tensor(
                    d3[:, hs_], mcp3[h][:], -1.0, y[:, hs_], OP.mult, OP.add
                )
            e3 = tmpA.tile([128, CH], F32, tag="tA")
            nc.vector.tensor_tensor(e3[:], d3[:], r3[:], OP.mult)
            hq = tmpCr.tile([128, CH], F16, tag="tC_r")
            nc.scalar.activation(
                hq[:], e3[:], AF.Relu,
                bias=small_vecs["q1_beta"][:], scale=small_vecs["q1_g"][:],
            )
            qsb = qpool.tile([NACT, CH], F32, tag="qsb")
            for h, hs_ in enumerate(HALVES):
                psq = psstat.tile([NACT, TILE], F32, tag="st")
                nc.tensor.matmul(psq[:], q2t_sb[:], hq[:, hs_], start=True, stop=True)
                nc.scalar.activation(
                    qsb[:, hs_], psq[:], AF.Identity, bias=small_vecs["q2_b"][:]
                )
            nc.sync.dma_start(q_t[:, cols], qsb[:])

        # ---- software-pipelined phase schedule: A(t) | C(t-2) | B(t-1)
        # (A and C share the rsqrt ACT table set; B's sigmoid set goes last
        # in each iteration so the ScalarE switches sets only twice per iter)
        for it in range(N_CHUNKS + 2):
            if it < N_CHUNKS:
                emit_A(it)
            if it >= 2:
                emit_C(it - 2)
            if 1 <= it <= N_CHUNKS:
                emit_B(it - 1)

        for p in reversed(ctx_pools):
            p.__exit__(None, None, None)

    _split_multi_waits(nc, mybir)
    return nc


def _get_nc():
    if "nc" not in _CACHE:
        _CACHE["nc"] = _build_nc()
    return _CACHE["nc"]


# --------------------------------------------------------------------------
# numpy fallback (general h0/c0 path; bit-faithful to the reference math)
# --------------------------------------------------------------------------
def _numpy_reference(inp):
    p = {k: np.asarray(v, np.float32) for k, v in inp.items()}
    obs, h0, c0 = p["obs"], p["h0"], p["c0"]
    Bx = obs.shape[0]

    def ln(x, g, b):
        m = x.mean(-1, keepdims=True)
        v = ((x - m) ** 2).mean(-1, keepdims=True)
        return (x - m) / np.sqrt(v + EPS) * g + b

    grid = obs[:, :25].reshape(Bx, 1, 5, 5)
    pos, met, aff = obs[:, 25:27], obs[:, 27:35], obs[:, 35:50]

    def conv(x, w, b):
        Co, Ci = w.shape[0], w.shape[1]
        xp = np.zeros((Bx, Ci, 7, 7), np.float32)
        xp[:, :, 1:6, 1:6] = x
        out = np.zeros((Bx, Co, 5, 5), np.float32)
        for kh in range(3):
            for kw in range(3):
                patch = xp[:, :, kh : kh + 5, kw : kw + 5]
                out += np.einsum("bchw,oc->bohw", patch, w[:, :, kh, kw])
        return out + b[None, :, None, None]

    x = np.maximum(conv(grid, p["conv1_w"], p["conv1_b"]), 0)
    x = np.maximum(conv(x, p["conv2_w"], p["conv2_b"]), 0)
    x = x.reshape(Bx, -1)
    vision = np.maximum(
        ln(x @ p["vis_w"].T + p["vis_b"], p["vis_g"], p["vis_beta"]), 0
    )
    pf = np.maximum(pos @ p["pos_w"].T + p["pos_b"], 0)
    mf = np.maximum(met @ p["met_w"].T + p["met_b"], 0)
    af = np.maximum(aff @ p["aff_w"].T + p["aff_b"], 0)
    comb = np.concatenate([vision, pf, mf, af], axis=1)
    gates = comb @ p["w_ih"].T + p["b_ih"] + h0 @ p["w_hh"].T + p["b_hh"]
    i, f, g, o = np.split(gates, 4, axis=1)
    sig = lambda v: 1.0 / (1.0 + np.exp(-v))
    i, f, g, o = sig(i), sig(f), np.tanh(g), sig(o)
    c_new = f * c0 + i * g
    h_new = o * np.tanh(c_new)
    lstm_out = ln(h_new, p["ln_g"], p["ln_b"])
    hq = np.maximum(
        ln(lstm_out @ p["q1_w"].T + p["q1_b"], p["q1_g"], p["q1_beta"]), 0
    )
    qv = hq @ p["q2_w"].T + p["q2_b"]
    return qv.astype(np.float32), h_new.astype(np.float32), c_new.astype(np.float32)


# --------------------------------------------------------------------------
# entry point
# --------------------------------------------------------------------------
MM_PARAMS = {"w0", "k2", "vist", "wiht", "w2t", "q2t", "ones128", "ones256", "negones128"}


def run_sharded(inputs, trace=False):
    """Build per-core input maps, run the SPMD kernel, return (results, bench)."""
    from concourse.bass_utils import run_bass_kernel_spmd

    w = _prep_weights(inputs)
    for k in MM_PARAMS:
        w[k] = np.ascontiguousarray(w[k].astype(np.float16))
    obs = np.asarray(inputs["obs"], np.float32)
    obs_t = np.ascontiguousarray(obs.T.astype(np.float16))  # [50, B] fp16

    in_maps = []
    for ci in range(N_CORES):
        m = {k: w[k] for k in WEIGHT_SHAPES}
        m["obs_t"] = np.ascontiguousarray(obs_t[:, ci * B_CORE : (ci + 1) * B_CORE])
        in_maps.append(m)

    nc = _get_nc()
    res = run_bass_kernel_spmd(
        nc, in_maps, list(range(N_CORES)), trace=trace
    )
    return res


def kernel(**inputs):
    h0 = np.asarray(inputs["h0"])
    c0 = np.asarray(inputs["c0"])
    if np.any(h0) or np.any(c0):
        return _numpy_reference(inputs)

    res = run_sharded(inputs, trace=False)

    q = np.empty((B, NACT), np.float32)
    h = np.empty((B, HID), np.float32)
    c = np.empty((B, HID), np.float32)
    for ci in range(N_CORES):
        sl = slice(ci * B_CORE, (ci + 1) * B_CORE)
        out = res.results[ci]
        q[sl] = out["q_t"].T
        h[sl] = out["h_t"].T
        c[sl] = out["c_t"].T
    return q, h, c


# revision 25
# speedup vs baseline: 1.2572x; 1.2572x over previous
"""Trainium2 Bass kernel for nn_RecurrentSpatialQNetwork.

Strategy (pure data parallel over 8 NeuronCores, batch 65536 -> 8192/core):
- Feature-major on-chip layout: activations stored [features(partitions), batch(free)],
  batch processed in 16 tiles of 512 columns per core.
- conv1 + pos/met/aff encoders fused into one [50 x 496] Toeplitz matmul.
- conv2 as dense [400 x 800] Toeplitz matmul.
- All matmuls in float32r (full-rate fp32 on the PE at N=512).
- LayerNorm stats via ones-matrix matmuls that produce mean/meansq already
  broadcast across partitions ([128,512] PSUM tiles).
- h0 = c0 = 0 exploited (spec fill=zeros): w_hh matmul and the forget gate are
  dropped; LN_h is folded into the q1 matmul. A numpy fallback handles the
  general (nonzero h0/c0) case exactly.
- Three phases per core (encoder / LSTM / Q-head) so the ScalarEngine only
  switches activation-table sets twice (sqrt set -> sigmoid set -> sqrt set).
"""
import os
import sys

for _p in ("/opt/trn_rl_repo", "/root/.axon_site/_ro/trn_rl_repo"):
    if os.path.isdir(_p) and _p not in sys.path:
        sys.path.append(_p)

import numpy as np

B = 65536
WIN, POS, MET, AFF, HID, NACT = 5, 2, 8, 15, 256, 8
N_CORES = 8
B_CORE = B // N_CORES  # 8192
TILE = 512
N_TILES = B_CORE // TILE  # 16
EPS = 1e-5

_CACHE = {}


# --------------------------------------------------------------------------
# host-side weight preparation
# --------------------------------------------------------------------------
def _prep_weights(p):
    f32 = np.float32
    conv1_w, conv2_w = np.asarray(p["conv1_w"]), np.asarray(p["conv2_w"])
    W0 = np.zeros((50, 496), f32)
    for co in range(16):
        for r in range(WIN):
            for c in range(WIN):
                o = co * 25 + r * 5 + c
                for dr in (-1, 0, 1):
                    for dc in (-1, 0, 1):
                        rr, cc = r + dr, c + dc
                        if 0 <= rr < 5 and 0 <= cc < 5:
                            W0[rr * 5 + cc, o] = conv1_w[co, 0, dr + 1, dc + 1]
    W0[25:27, 400:432] = p["pos_w"].T
    W0[27:35, 432:464] = p["met_w"].T
    W0[35:50, 464:496] = p["aff_w"].T
    b0 = np.concatenate(
        [np.repeat(p["conv1_b"], 25), p["pos_b"], p["met_b"], p["aff_b"]]
    ).astype(f32)

    K2 = np.zeros((400, 800), f32)
    for co in range(32):
        for ci in range(16):
            for r in range(WIN):
                for c in range(WIN):
                    o = co * 25 + r * 5 + c
                    for dr in (-1, 0, 1):
                        for dc in (-1, 0, 1):
                            rr, cc = r + dr, c + dc
                            if 0 <= rr < 5 and 0 <= cc < 5:
                                K2[ci * 25 + rr * 5 + cc, o] = conv2_w[
                                    co, ci, dr + 1, dc + 1
                                ]
    b2 = np.repeat(p["conv2_b"], 25).astype(f32)

    igo = np.r_[0:256, 512:1024]
    W2 = p["q1_w"] * p["ln_g"][None, :]
    return dict(
        w0=np.ascontiguousarray(W0),
        k2=np.ascontiguousarray(K2),
        vist=np.ascontiguousarray(p["vis_w"].T.astype(f32)),
        wiht=np.ascontiguousarray(p["w_ih"][igo, :].T.astype(f32)),
        w2t=np.ascontiguousarray(W2.T.astype(f32)),
        q2t=np.ascontiguousarray(p["q2_w"].T.astype(f32)),
        b0=b0.reshape(-1, 1),
        b2=b2.reshape(-1, 1),
        bg=(p["b_ih"] + p["b_hh"])[igo].astype(f32).reshape(-1, 1),
        w0col=(p["q1_w"] @ p["ln_b"] + p["q1_b"]).astype(f32).reshape(-1, 1),
        negv=(-W2.sum(axis=1)).astype(f32).reshape(-1, 1),
        vis_g=np.asarray(p["vis_g"], f32).reshape(-1, 1),
        vis_beta=np.asarray(p["vis_beta"], f32).reshape(-1, 1),
        q1_g=np.asarray(p["q1_g"], f32).reshape(-1, 1),
        q1_beta=np.asarray(p["q1_beta"], f32).reshape(-1, 1),
        q2_b=np.asarray(p["q2_b"], f32).reshape(-1, 1),
        ones128=np.full((128, 128), 1.0 / 128.0, f32),
        ones256=np.full((128, 128), 1.0 / 256.0, f32),
        negones128=np.full((128, 128), -1.0 / 128.0, f32),
    )


WEIGHT_SHAPES = {
    "w0": [50, 496],
    "k2": [400, 800],
    "vist": [800, 128],
    "wiht": [224, 768],
    "w2t": [256, 128],
    "q2t": [128, 8],
    "b0": [496, 1],
    "b2": [800, 1],
    "bg": [768, 1],
    "w0col": [128, 1],
    "negv": [128, 1],
    "vis_g": [128, 1],
    "vis_beta": [128, 1],
    "q1_g": [128, 1],
    "q1_beta": [128, 1],
    "q2_b": [8, 1],
    "ones128": [128, 128],
    "ones256": [128, 128],
    "negones128": [128, 128],
}


# --------------------------------------------------------------------------
# walrus workaround: this container's walrus accepts only ONE sync wait per
# instruction; split extras into preceding same-engine NoOps.
# --------------------------------------------------------------------------
def _split_multi_waits(nc, mybir):
    n = 0
    for f in nc.m.functions:
        for blk in f.blocks:
            out = []
            changed = False
            for inst in blk.instructions:
                si = inst.sync_info
                if si is not None and len(si.on_wait) > 1:
                    waits = list(si.on_wait)
                    for j, w in enumerate(waits[:-1]):
                        nop = mybir.InstNoOp(name=f"{inst.name}-wsplit{j}")
                        nop.engine = inst.engine
                        nop.sync_info = mybir.SyncInfo(on_wait=[w], on_update=[])
                        out.append(nop)
                        n += 1
                    inst.sync_info = mybir.SyncInfo(
                        on_wait=[waits[-1]], on_update=list(si.on_update)
                    )
                    changed = True
                out.append(inst)
            if changed:
                blk.instructions = out
    return n


# --------------------------------------------------------------------------
# bass kernel construction
# --------------------------------------------------------------------------
def _build_nc():
    import concourse.bass as bass
    import concourse.tile as tile
    from concourse import mybir

    dt = mybir.dt
    F32, F16 = dt.float32, dt.float16
    AF = mybir.ActivationFunctionType
    OP = mybir.AluOpType
    CH = 2 * TILE  # 1024-wide chunks, two 512 matmul halves
    N_CHUNKS = B_CORE // CH  # 8

    nc = bass.Bass()

    def act_rsqrt(out, in_, bias_ap):
        eng = nc.scalar
        ins = [
            eng.lower_ap(in_),
            eng.lower_ap(bias_ap),
            mybir.ImmediateValue(dtype=F32, value=1.0),
            mybir.ImmediateValue(dtype=F32, value=0.0),
        ]
        return eng.add_instruction(
            mybir.InstActivation(
                name=nc.get_next_instruction_name(),
                func=AF.Rsqrt,
                ins=ins,
                outs=[eng.lower_ap(out)],
            )
        )

    MM_P = {"w0", "k2", "vist", "wiht", "w2t", "q2t", "ones128", "ones256",
            "negones128"}
    obs_t = nc.declare_dram_parameter("obs_t", [50, B_CORE], F16, isOutput=False)
    wd = {
        k: nc.declare_dram_parameter(k, shp, F16 if k in MM_P else F32, isOutput=False)
        for k, shp in WEIGHT_SHAPES.items()
    }
    q_t = nc.declare_dram_parameter("q_t", [NACT, B_CORE], F32, isOutput=True)
    h_t = nc.declare_dram_parameter("h_t", [HID, B_CORE], F32, isOutput=True)
    c_t = nc.declare_dram_parameter("c_t", [HID, B_CORE], F32, isOutput=True)

    with tile.TileContext(nc) as tc:
        ctx_pools = []

        def pool(name, bufs, space="SBUF"):
            p = tc.tile_pool(name=name, bufs=bufs, space=space)
            ctx_pools.append(p)
            return p.__enter__()

        singles = pool("singles", 1)
        w0_sb = singles.tile([50, 496], F16, tag="w0")
        nc.sync.dma_start(w0_sb[:], wd["w0"][:])
        k2_sb = []
        for j, kn in enumerate((128, 128, 128, 16)):
            t = singles.tile([kn, 800], F16, tag=f"k2_{j}")
            nc.sync.dma_start(t[:], wd["k2"][j * 128 : j * 128 + kn, :])
            k2_sb.append(t)
        vis_sb = []
        for j in range(7):
            kn = 128 if j < 6 else 32
            t = singles.tile([kn, 128], F16, tag=f"vis_{j}")
            nc.sync.dma_start(t[:], wd["vist"][j * 128 : j * 128 + kn, :])
            vis_sb.append(t)
        wih_sb = []
        for j, kn in enumerate((128, 96)):
            t = singles.tile([kn, 768], F16, tag=f"wih_{j}")
            nc.sync.dma_start(t[:], wd["wiht"][j * 128 : j * 128 + kn, :])
            wih_sb.append(t)
        w2t_sb = []
        for j in range(2):
            t = singles.tile([128, 128], F16, tag=f"w2t_{j}")
            nc.sync.dma_start(t[:], wd["w2t"][j * 128 : (j + 1) * 128, :])
            w2t_sb.append(t)
        q2t_sb = singles.tile([128, NACT], F16, tag="q2t")
        nc.sync.dma_start(q2t_sb[:], wd["q2t"][:])
        ones128 = singles.tile([128, 128], F16, tag="ones128")
        nc.sync.dma_start(ones128[:], wd["ones128"][:])
        ones256 = singles.tile([128, 128], F16, tag="ones256")
        nc.sync.dma_start(ones256[:], wd["ones256"][:])
        negones128 = singles.tile([128, 128], F16, tag="negones128")
        nc.sync.dma_start(negones128[:], wd["negones128"][:])

        b0_sb = []
        for j, (p0, pn) in enumerate(((0, 128), (128, 128), (256, 128), (384, 16), (400, 96))):
            t = singles.tile([pn, 1], F32, tag=f"b0_{j}")
            nc.sync.dma_start(t[:], wd["b0"][p0 : p0 + pn, :])
            b0_sb.append(t)
        b2_sb = []
        for j in range(7):
            pn = 128 if j < 6 else 32
            t = singles.tile([pn, 1], F32, tag=f"b2_{j}")
            nc.sync.dma_start(t[:], wd["b2"][j * 128 : j * 128 + pn, :])
            b2_sb.append(t)
        bg_sb = []
        for j in range(6):
            t = singles.tile([128, 1], F32, tag=f"bg_{j}")
            nc.sync.dma_start(t[:], wd["bg"][j * 128 : (j + 1) * 128, :])
            bg_sb.append(t)
        small_vecs = {}
        for name, pn in (
            ("negv", 128),
            ("w0col", 128),
            ("vis_g", 128),
            ("vis_beta", 128),
            ("q1_g", 128),
            ("q1_beta", 128),
            ("q2_b", 8),
        ):
            t = singles.tile([pn, 1], F32, tag=f"sv_{name}")
            nc.sync.dma_start(t[:], wd[name][:])
            small_vecs[name] = t
        eps_sb = singles.tile([128, 1], F32, tag="epsvec")
        nc.vector.memset(eps_sb[:], EPS)

        # ---- pools
        xpool = pool("x0", 2)
        a1pool = pool("a1", 5)
        zpool = pool("z", 8)
        tmpA = pool("tmpA", 9)
        sqpool = pool("sq", 4)
        xspool = pool("xs", 2)
        stspool = pool("sts", 8)
        vopool = pool("vo", 4)       # vision tiles passed A->B
        e96pool = pool("e96", 4)     # enc96 tiles passed A->B
        gpool = pool("gates", 4)
        chpool = pool("ch", 4)       # c + tanh_c f32 temps
        hpool = pool("hh", 4)        # h fp16 tiles passed B->C
        tmpCr = pool("tmpCr", 6)
        qpool = pool("qout", 2)
        psbig = pool("psbig", 3, space="PSUM")
        psstat = pool("psstat", 2, space="PSUM")

        HALVES = (slice(0, TILE), slice(TILE, CH))

        vis_tiles = {}
        enc_tiles = {}
        h_tiles = {}

        # ---------------- phase emitters ----------------
        def emit_A(t):
            cols = slice(t * CH, (t + 1) * CH)
            x0 = xpool.tile([50, CH], F16, tag="x0")
            nc.sync.dma_start(x0[:], obs_t[:, cols])

            a1 = []
            enc96 = e96pool.tile([96, CH], F16, tag="enc96")
            for j, (m0, mn) in enumerate(
                ((0, 128), (128, 128), (256, 128), (384, 16), (400, 96))
            ):
                ps = psbig.tile([mn, CH], F32, tag="ps")
                for h, hs_ in enumerate(HALVES):
                    nc.tensor.matmul(
                        ps[:, hs_], w0_sb[:, m0 : m0 + mn], x0[:, hs_],
                        start=True, stop=True,
                    )
                if j < 3:
                    dst = a1pool.tile([128, CH], F16, tag="a1")
                    if j % 2 == 0:
                        nc.scalar.activation(dst[:], ps[:], AF.Relu, bias=b0_sb[j][:])
                    else:
                        nc.vector.tensor_scalar(
                            dst[:], ps[:], b0_sb[j][:], 0.0, OP.add, OP.max
                        )
                    a1.append(dst)
                elif j == 3:
                    g16 = xpool.tile([16, CH], F16, tag="g16")
                    nc.vector.tensor_scalar(
                        g16[:], ps[:], b0_sb[3][:], 0.0, OP.add, OP.max
                    )
                    a1.append(g16)
                else:
                    nc.scalar.activation(enc96[:], ps[:], AF.Relu, bias=b0_sb[4][:])
            enc_tiles[t] = enc96

            z = []
            for j in range(7):
                m0, mn = j * 128, (128 if j < 6 else 32)
                ps = psbig.tile([mn, CH], F32, tag="ps")
                for h, hs_ in enumerate(HALVES):
                    for kj in range(4):
                        nc.tensor.matmul(
                            ps[:, hs_], k2_sb[kj][:, m0 : m0 + mn], a1[kj][:, hs_],
                            start=(kj == 0), stop=(kj == 3),
                        )
                dst = zpool.tile([mn, CH], F16, tag="z")
                if j % 2 == 0:
                    nc.vector.tensor_scalar(
                        dst[:], ps[:], b2_sb[j][:], 0.0, OP.add, OP.max
                    )
                else:
                    nc.scalar.activation(dst[:], ps[:], AF.Relu, bias=b2_sb[j][:])
                z.append(dst)

            psv = psbig.tile([128, CH], F32, tag="ps")
            for h, hs_ in enumerate(HALVES):
                for kj in range(7):
                    nc.tensor.matmul(
                        psv[:, hs_], vis_sb[kj][:], z[kj][:, hs_],
                        start=(kj == 0), stop=False,
                    )
            # center-in-PSUM LN
            x_s = xspool.tile([128, CH], F16, tag="xs")
            nc.vector.tensor_copy(x_s[:], psv[:])
            for h, hs_ in enumerate(HALVES):
                nc.tensor.matmul(
                    psv[:, hs_], negones128[:], x_s[:, hs_], start=False, stop=True
                )
            xcsq = sqpool.tile([128, CH], F16, tag="sq")
            nc.scalar.activation(xcsq[:], psv[:], AF.Square)
            rr = tmpA.tile([128, CH], F32, tag="tA")
            for h, hs_ in enumerate(HALVES):
                qb = psstat.tile([128, TILE], F32, tag="st")
                nc.tensor.matmul(qb[:], ones128[:], xcsq[:, hs_], start=True, stop=True)
                act_rsqrt(rr[:, hs_], qb[:], eps_sb[:])
            e = tmpA.tile([128, CH], F32, tag="tA")
            nc.vector.tensor_tensor(e[:], psv[:], rr[:], OP.mult)
            vis = vopool.tile([128, CH], F16, tag="visout")
            nc.scalar.activation(
                vis[:], e[:], AF.Relu,
                bias=small_vecs["vis_beta"][:], scale=small_vecs["vis_g"][:],
            )
            vis_tiles[t] = vis

        GATE_FN = (AF.Sigmoid, AF.Sigmoid, AF.Tanh, AF.Tanh, AF.Sigmoid, AF.Sigmoid)

        def emit_B(t):
            cols = slice(t * CH, (t + 1) * CH)
            vis = vis_tiles.pop(t)
            enc96 = enc_tiles.pop(t)
            gates = []
            for mj in range(6):
                ps = psbig.tile([128, CH], F32, tag="ps")
                for h, hs_ in enumerate(HALVES):
                    nc.tensor.matmul(
                        ps[:, hs_], wih_sb[0][:, mj * 128 : (mj + 1) * 128],
                        vis[:, hs_], start=True, stop=False,
                    )
                    nc.tensor.matmul(
                        ps[:, hs_], wih_sb[1][:, mj * 128 : (mj + 1) * 128],
                        enc96[:, hs_], start=False, stop=True,
                    )
                gt = gpool.tile([128, CH], F32, tag="gate")
                nc.scalar.activation(gt[:], ps[:], GATE_FN[mj], bias=bg_sb[mj][:])
                gates.append(gt)
            hts = []
            for j in range(2):
                cj = chpool.tile([128, CH], F32, tag="ch")
                nc.vector.tensor_tensor(cj[:], gates[j][:], gates[2 + j][:], OP.mult)
                nc.sync.dma_start(c_t[j * 128 : (j + 1) * 128, cols], cj[:])
                tcj = chpool.tile([128, CH], F32, tag="ch")
                nc.scalar.activation(tcj[:], cj[:], AF.Tanh)
                hj = chpool.tile([128, CH], F32, tag="ch32")
                nc.vector.tensor_tensor(hj[:], gates[4 + j][:], tcj[:], OP.mult)
                nc.sync.dma_start(h_t[j * 128 : (j + 1) * 128, cols], hj[:])
                h16 = hpool.tile([128, CH], F16, tag="hh")
                nc.vector.tensor_copy(h16[:], hj[:])
                hts.append(h16)
            h_tiles[t] = hts

        def emit_C(t):
            cols = slice(t * CH, (t + 1) * CH)
            hs = h_tiles.pop(t)
            hsq = []
            for j in range(2):
                s = sqpool.tile([128, CH], F16, tag="sq")
                nc.scalar.activation(s[:], hs[j][:], AF.Square)
                hsq.append(s)
            r2 = tmpA.tile([128, CH], F32, tag="tA")
            mcp2 = []
            for h, hs_ in enumerate(HALVES):
                mb = psstat.tile([128, TILE], F32, tag="st")
                for kb in range(2):
                    nc.tensor.matmul(
                        mb[:], ones256[:], hs[kb][:, hs_],
                        start=(kb == 0), stop=(kb == 1),
                    )
                mcp = stspool.tile([128, TILE], F32, tag="sts")
                nc.vector.tensor_copy(mcp[:], mb[:])
                mcp2.append(mcp)
                qb = psstat.tile([128, TILE], F32, tag="st")
                for kb in range(2):
                    nc.tensor.matmul(
                        qb[:], ones256[:], hsq[kb][:, hs_],
                        start=(kb == 0), stop=(kb == 1),
                    )
                msq = stspool.tile([128, TILE], F32, tag="sts")
                nc.scalar.activation(msq[:], mcp[:], AF.Square)
                rin = stspool.tile([128, TILE], F32, tag="sts")
                nc.vector.scalar_tensor_tensor(
                    rin[:], qb[:], 1.0, msq[:], OP.mult, OP.subtract
                )
                act_rsqrt(r2[:, hs_], rin[:], eps_sb[:])
            psu = psbig.tile([128, CH], F32, tag="ps")
            for h, hs_ in enumerate(HALVES):
                nc.tensor.matmul(
                    psu[:, hs_], w2t_sb[0][:], hs[0][:, hs_], start=True, stop=False
                )
                nc.tensor.matmul(
                    psu[:, hs_], w2t_sb[1][:], hs[1][:, hs_], start=False, stop=True
                )
            y1 = tmpA.tile([128, CH], F32, tag="tA")
            mr = tmpA.tile([128, CH], F32, tag="tA")
            for h, hs_ in enumerate(HALVES):
                nc.vector.scalar_tensor_tensor(
                    y1[:, hs_], psu[:, hs_], small_vecs["w0col"][:], r2[:, hs_],
                    OP.add, OP.mult,
                )
                nc.vector.tensor_tensor(mr[:, hs_], mcp2[h][:], r2[:, hs_], OP.mult)
            y = tmpCr.tile([128, CH], F16, tag="tC_r")
            nc.vector.scalar_tensor_tensor(
                y[:], mr[:], small_vecs["negv"][:], y1[:], OP.mult, OP.add
            )
            ysq = sqpool.tile([128, CH], F16, tag="sq")
            nc.scalar.activation(ysq[:], y[:], AF.Square)
            r3 = tmpA.tile([128, CH], F32, tag="tA")
            mcp3 = []
            for h, hs_ in enumerate(HALVES):
                mb = psstat.tile([128, TILE], F32, tag="st")
                nc.tensor.matmul(mb[:], ones128[:], y[:, hs_], start=True, stop=True)
                mcp = stspool.tile([128, TILE], F32, tag="sts")
                nc.vector.tensor_copy(mcp[:], mb[:])
                mcp3.append(mcp)
                qb = psstat.tile([128, TILE], F32, tag="st")
                nc.tensor.matmul(qb[:], ones128[:], ysq[:, hs_], start=True, stop=True)
                msq = stspool.tile([128, TILE], F32, tag="sts")
                nc.scalar.activation(msq[:], mcp[:], AF.Square)
                rin = stspool.tile([128, TILE], F32, tag="sts")
                nc.vector.scalar_tensor_tensor(
                    rin[:], qb[:], 1.0, msq[:], OP.mult, OP.subtract
                )
                act_rsqrt(r3[:, hs_], rin[:], eps_sb[:])
            d3 = tmpA.tile([128, CH], F32, tag="tA")
            for h, hs_ in enumerate(HALVES):
                nc.vector.scalar_tensor_tensor(
                    d3[:, hs_], mcp3[h][:], -1.0, y[:, hs_], OP.mult, OP.add
                )
            e3 = tmpA.tile([128, CH], F32, tag="tA")
            nc.vector.tensor_tensor(e3[:], d3[:], r3[:], OP.mult)
            hq = tmpCr.tile([128, CH], F16, tag="tC_r")
            nc.scalar.activation(
                hq[:], e3[:], AF.Relu,
                bias=small_vecs["q1_beta"][:], scale=small_vecs["q1_g"][:],
            )
            qsb = qpool.tile([NACT, CH], F32, tag="qsb")
            for h, hs_ in enumerate(HALVES):
                psq = psstat.tile([NACT, TILE], F32, tag="st")
                nc.tensor.matmul(psq[:], q2t_sb[:], hq[:, hs_], start=True, stop=True)
                nc.scalar.activation(
                    qsb[:, hs_], psq[:], AF.Identity, bias=small_vecs["q2_b"][:]
                )
            nc.sync.dma_start(q_t[:, cols], qsb[:])

        # ---- software-pipelined phase schedule: A(t) | C(t-2) | B(t-1)
        # (A and C share the rsqrt ACT table set; B's sigmoid set goes last
        # in each iteration so the ScalarE switches sets only twice per iter)
        for it in range(N_CHUNKS + 2):
            if it < N_CHUNKS:
                emit_A(it)
            if it >= 2:
                emit_C(it - 2)
            if 1 <= it <= N_CHUNKS:
                emit_B(it - 1)

        for p in reversed(ctx_pools):
            p.__exit__(None, None, None)

    _split_multi_waits(nc, mybir)
    return nc


def _get_nc():
    if "nc" not in _CACHE:
        _CACHE["nc"] = _build_nc()
    return _CACHE["nc"]


# --------------------------------------------------------------------------
# numpy fallback (general h0/c0 path; bit-faithful to the reference math)
# --------------------------------------------------------------------------
def _numpy_reference(inp):
    p = {k: np.asarray(v, np.float32) for k, v in inp.items()}
    obs, h0, c0 = p["obs"], p["h0"], p["c0"]
    Bx = obs.shape[0]

    def ln(x, g, b):
        m = x.mean(-1, keepdims=True)
        v = ((x - m) ** 2).mean(-1, keepdims=True)
        return (x - m) / np.sqrt(v + EPS) * g + b

    grid = obs[:, :25].reshape(Bx, 1, 5, 5)
    pos, met, aff = obs[:, 25:27], obs[:, 27:35], obs[:, 35:50]

    def conv(x, w, b):
        Co, Ci = w.shape[0], w.shape[1]
        xp = np.zeros((Bx, Ci, 7, 7), np.float32)
        xp[:, :, 1:6, 1:6] = x
        out = np.zeros((Bx, Co, 5, 5), np.float32)
        for kh in range(3):
            for kw in range(3):
                patch = xp[:, :, kh : kh + 5, kw : kw + 5]
                out += np.einsum("bchw,oc->bohw", patch, w[:, :, kh, kw])
        return out + b[None, :, None, None]

    x = np.maximum(conv(grid, p["conv1_w"], p["conv1_b"]), 0)
    x = np.maximum(conv(x, p["conv2_w"], p["conv2_b"]), 0)
    x = x.reshape(Bx, -1)
    vision = np.maximum(
        ln(x @ p["vis_w"].T + p["vis_b"], p["vis_g"], p["vis_beta"]), 0
    )
    pf = np.maximum(pos @ p["pos_w"].T + p["pos_b"], 0)
    mf = np.maximum(met @ p["met_w"].T + p["met_b"], 0)
    af = np.maximum(aff @ p["aff_w"].T + p["aff_b"], 0)
    comb = np.concatenate([vision, pf, mf, af], axis=1)
    gates = comb @ p["w_ih"].T + p["b_ih"] + h0 @ p["w_hh"].T + p["b_hh"]
    i, f, g, o = np.split(gates, 4, axis=1)
    sig = lambda v: 1.0 / (1.0 + np.exp(-v))
    i, f, g, o = sig(i), sig(f), np.tanh(g), sig(o)
    c_new = f * c0 + i * g
    h_new = o * np.tanh(c_new)
    lstm_out = ln(h_new, p["ln_g"], p["ln_b"])
    hq = np.maximum(
        ln(lstm_out @ p["q1_w"].T + p["q1_b"], p["q1_g"], p["q1_beta"]), 0
    )
    qv = hq @ p["q2_w"].T + p["q2_b"]
    return qv.astype(np.float32), h_new.astype(np.float32), c_new.astype(np.float32)


# --------------------------------------------------------------------------
# entry point
# --------------------------------------------------------------------------
MM_PARAMS = {"w0", "k2", "vist", "wiht", "w2t", "q2t", "ones128", "ones256", "negones128"}


def run_sharded(inputs, trace=False):
    """Build per-core input maps, run the SPMD kernel, return (results, bench)."""
    from concourse.bass_utils import run_bass_kernel_spmd

    w = _prep_weights(inputs)
    for k in MM_PARAMS:
        w[k] = np.ascontiguousarray(w[k].astype(np.float16))
    obs = np.asarray(inputs["obs"], np.float32)
    obs_t = np.ascontiguousarray(obs.T.astype(np.float16))  # [50, B] fp16

    in_maps = []
    for ci in range(N_CORES):
        m = {k: w[k] for k in WEIGHT_SHAPES}
        m["obs_t"] = np.ascontiguousarray(obs_t[:, ci * B_CORE : (ci + 1) * B_CORE])
        in_maps.append(m)

    nc = _get_nc()
    res = run_bass_kernel_spmd(
        nc, in_maps, list(range(N_CORES)), trace=trace
    )
    return res


def kernel(**inputs):
    h0 = np.asarray(inputs["h0"])
    c0 = np.asarray(inputs["c0"])
    if np.any(h0) or np.any(c0):
        return _numpy_reference(inputs)

    res = run_sharded(inputs, trace=False)

    q = np.empty((B, NACT), np.float32)
    h = np.empty((B, HID), np.float32)
    c = np.empty((B, HID), np.float32)
    for ci in range(N_CORES):
        sl = slice(ci * B_CORE, (ci + 1) * B_CORE)
        out = res.results[ci]
        q[sl] = out["q_t"].T
        h[sl] = out["h_t"].T
        c[sl] = out["c_t"].T
    return q, h, c


# revision 26
# speedup vs baseline: 1.3305x; 1.0583x over previous
"""Trainium2 Bass kernel for nn_RecurrentSpatialQNetwork.

Strategy (pure data parallel over 8 NeuronCores, batch 65536 -> 8192/core):
- Feature-major on-chip layout: activations stored [features(partitions), batch(free)],
  batch processed in 16 tiles of 512 columns per core.
- conv1 + pos/met/aff encoders fused into one [50 x 496] Toeplitz matmul.
- conv2 as dense [400 x 800] Toeplitz matmul.
- All matmuls in float32r (full-rate fp32 on the PE at N=512).
- LayerNorm stats via ones-matrix matmuls that produce mean/meansq already
  broadcast across partitions ([128,512] PSUM tiles).
- h0 = c0 = 0 exploited (spec fill=zeros): w_hh matmul and the forget gate are
  dropped; LN_h is folded into the q1 matmul. A numpy fallback handles the
  general (nonzero h0/c0) case exactly.
- Three phases per core (encoder / LSTM / Q-head) so the ScalarEngine only
  switches activation-table sets twice (sqrt set -> sigmoid set -> sqrt set).
"""
import os
import sys

for _p in ("/opt/trn_rl_repo", "/root/.axon_site/_ro/trn_rl_repo"):
    if os.path.isdir(_p) and _p not in sys.path:
        sys.path.append(_p)

import numpy as np

B = 65536
WIN, POS, MET, AFF, HID, NACT = 5, 2, 8, 15, 256, 8
N_CORES = 8
B_CORE = B // N_CORES  # 8192
TILE = 512
N_TILES = B_CORE // TILE  # 16
EPS = 1e-5

_CACHE = {}


# --------------------------------------------------------------------------
# host-side weight preparation
# --------------------------------------------------------------------------
def _prep_weights(p):
    f32 = np.float32
    conv1_w, conv2_w = np.asarray(p["conv1_w"]), np.asarray(p["conv2_w"])
    W0 = np.zeros((50, 496), f32)
    for co in range(16):
        for r in range(WIN):
            for c in range(WIN):
                o = co * 25 + r * 5 + c
                for dr in (-1, 0, 1):
                    for dc in (-1, 0, 1):
                        rr, cc = r + dr, c + dc
                        if 0 <= rr < 5 and 0 <= cc < 5:
                            W0[rr * 5 + cc, o] = conv1_w[co, 0, dr + 1, dc + 1]
    W0[25:27, 400:432] = p["pos_w"].T
    W0[27:35, 432:464] = p["met_w"].T
    W0[35:50, 464:496] = p["aff_w"].T
    b0 = np.concatenate(
        [np.repeat(p["conv1_b"], 25), p["pos_b"], p["met_b"], p["aff_b"]]
    ).astype(f32)

    K2 = np.zeros((400, 800), f32)
    for co in range(32):
        for ci in range(16):
            for r in range(WIN):
                for c in range(WIN):
                    o = co * 25 + r * 5 + c
                    for dr in (-1, 0, 1):
                        for dc in (-1, 0, 1):
                            rr, cc = r + dr, c + dc
                            if 0 <= rr < 5 and 0 <= cc < 5:
                                K2[ci * 25 + rr * 5 + cc, o] = conv2_w[
                                    co, ci, dr + 1, dc + 1
                                ]
    b2 = np.repeat(p["conv2_b"], 25).astype(f32)

    igo = np.r_[0:256, 512:1024]
    W2 = p["q1_w"] * p["ln_g"][None, :]
    return dict(
        w0=np.ascontiguousarray(W0),
        k2=np.ascontiguousarray(K2),
        vist=np.ascontiguousarray(p["vis_w"].T.astype(f32)),
        wiht=np.ascontiguousarray(p["w_ih"][igo, :].T.astype(f32)),
        w2t=np.ascontiguousarray(W2.T.astype(f32)),
        q2t=np.ascontiguousarray(p["q2_w"].T.astype(f32)),
        b0=b0.reshape(-1, 1),
        b2=b2.reshape(-1, 1),
        bg=(p["b_ih"] + p["b_hh"])[igo].astype(f32).reshape(-1, 1),
        w0col=(p["q1_w"] @ p["ln_b"] + p["q1_b"]).astype(f32).reshape(-1, 1),
        negv=(-W2.sum(axis=1)).astype(f32).reshape(-1, 1),
        vis_g=np.asarray(p["vis_g"], f32).reshape(-1, 1),
        vis_beta=np.asarray(p["vis_beta"], f32).reshape(-1, 1),
        q1_g=np.asarray(p["q1_g"], f32).reshape(-1, 1),
        q1_beta=np.asarray(p["q1_beta"], f32).reshape(-1, 1),
        q2_b=np.asarray(p["q2_b"], f32).reshape(-1, 1),
        ones128=np.full((128, 128), 1.0 / 128.0, f32),
        ones256=np.full((128, 128), 1.0 / 256.0, f32),
        negones128=np.full((128, 128), -1.0 / 128.0, f32),
    )


WEIGHT_SHAPES = {
    "w0": [50, 496],
    "k2": [400, 800],
    "vist": [800, 128],
    "wiht": [224, 768],
    "w2t": [256, 128],
    "q2t": [128, 8],
    "b0": [496, 1],
    "b2": [800, 1],
    "bg": [768, 1],
    "w0col": [128, 1],
    "negv": [128, 1],
    "vis_g": [128, 1],
    "vis_beta": [128, 1],
    "q1_g": [128, 1],
    "q1_beta": [128, 1],
    "q2_b": [8, 1],
    "ones128": [128, 128],
    "ones256": [128, 128],
    "negones128": [128, 128],
}


# --------------------------------------------------------------------------
# walrus workaround: this container's walrus accepts only ONE sync wait per
# instruction; split extras into preceding same-engine NoOps.
# --------------------------------------------------------------------------
def _split_multi_waits(nc, mybir):
    n = 0
    for f in nc.m.functions:
        for blk in f.blocks:
            out = []
            changed = False
            for inst in blk.instructions:
                si = inst.sync_info
                if si is not None and len(si.on_wait) > 1:
                    waits = list(si.on_wait)
                    for j, w in enumerate(waits[:-1]):
                        nop = mybir.InstNoOp(name=f"{inst.name}-wsplit{j}")
                        nop.engine = inst.engine
                        nop.sync_info = mybir.SyncInfo(on_wait=[w], on_update=[])
                        out.append(nop)
                        n += 1
                    inst.sync_info = mybir.SyncInfo(
                        on_wait=[waits[-1]], on_update=list(si.on_update)
                    )
                    changed = True
                out.append(inst)
            if changed:
                blk.instructions = out
    return n


# --------------------------------------------------------------------------
# bass kernel construction
# --------------------------------------------------------------------------
def _build_nc():
    import concourse.bass as bass
    import concourse.tile as tile
    from concourse import mybir

    dt = mybir.dt
    F32, F16 = dt.float32, dt.float16
    AF = mybir.ActivationFunctionType
    OP = mybir.AluOpType
    CH = 2 * TILE  # 1024-wide chunks, two 512 matmul halves
    N_CHUNKS = B_CORE // CH  # 8

    nc = bass.Bass()

    def act_rsqrt(out, in_, bias_ap):
        eng = nc.scalar
        ins = [
            eng.lower_ap(in_),
            eng.lower_ap(bias_ap),
            mybir.ImmediateValue(dtype=F32, value=1.0),
            mybir.ImmediateValue(dtype=F32, value=0.0),
        ]
        return eng.add_instruction(
            mybir.InstActivation(
                name=nc.get_next_instruction_name(),
                func=AF.Rsqrt,
                ins=ins,
                outs=[eng.lower_ap(out)],
            )
        )

    MM_P = {"w0", "k2", "vist", "wiht", "w2t", "q2t", "ones128", "ones256",
            "negones128"}
    obs_t = nc.declare_dram_parameter("obs_t", [50, B_CORE], F16, isOutput=False)
    wd = {
        k: nc.declare_dram_parameter(k, shp, F16 if k in MM_P else F32, isOutput=False)
        for k, shp in WEIGHT_SHAPES.items()
    }
    q_t = nc.declare_dram_parameter("q_t", [NACT, B_CORE], F32, isOutput=True)
    h_t = nc.declare_dram_parameter("h_t", [HID, B_CORE], F32, isOutput=True)
    c_t = nc.declare_dram_parameter("c_t", [HID, B_CORE], F32, isOutput=True)

    with tile.TileContext(nc) as tc:
        ctx_pools = []

        def pool(name, bufs, space="SBUF"):
            p = tc.tile_pool(name=name, bufs=bufs, space=space)
            ctx_pools.append(p)
            return p.__enter__()

        singles = pool("singles", 1)
        w0_sb = singles.tile([50, 496], F16, tag="w0")
        nc.sync.dma_start(w0_sb[:], wd["w0"][:])
        k2_sb = []
        for j, kn in enumerate((128, 128, 128, 16)):
            t = singles.tile([kn, 800], F16, tag=f"k2_{j}")
            nc.sync.dma_start(t[:], wd["k2"][j * 128 : j * 128 + kn, :])
            k2_sb.append(t)
        vis_sb = []
        for j in range(7):
            kn = 128 if j < 6 else 32
            t = singles.tile([kn, 128], F16, tag=f"vis_{j}")
            nc.sync.dma_start(t[:], wd["vist"][j * 128 : j * 128 + kn, :])
            vis_sb.append(t)
        wih_sb = []
        for j, kn in enumerate((128, 96)):
            t = singles.tile([kn, 768], F16, tag=f"wih_{j}")
            nc.sync.dma_start(t[:], wd["wiht"][j * 128 : j * 128 + kn, :])
            wih_sb.append(t)
        w2t_sb = []
        for j in range(2):
            t = singles.tile([128, 128], F16, tag=f"w2t_{j}")
            nc.sync.dma_start(t[:], wd["w2t"][j * 128 : (j + 1) * 128, :])
            w2t_sb.append(t)
        q2t_sb = singles.tile([128, NACT], F16, tag="q2t")
        nc.sync.dma_start(q2t_sb[:], wd["q2t"][:])
        ones128 = singles.tile([128, 128], F16, tag="ones128")
        nc.sync.dma_start(ones128[:], wd["ones128"][:])
        ones256 = singles.tile([128, 128], F16, tag="ones256")
        nc.sync.dma_start(ones256[:], wd["ones256"][:])
        negones128 = singles.tile([128, 128], F16, tag="negones128")
        nc.sync.dma_start(negones128[:], wd["negones128"][:])

        b0_sb = []
        for j, (p0, pn) in enumerate(((0, 128), (128, 128), (256, 128), (384, 16), (400, 96))):
            t = singles.tile([pn, 1], F32, tag=f"b0_{j}")
            nc.sync.dma_start(t[:], wd["b0"][p0 : p0 + pn, :])
            b0_sb.append(t)
        b2_sb = []
        for j in range(7):
            pn = 128 if j < 6 else 32
            t = singles.tile([pn, 1], F32, tag=f"b2_{j}")
            nc.sync.dma_start(t[:], wd["b2"][j * 128 : j * 128 + pn, :])
            b2_sb.append(t)
        bg_sb = []
        for j in range(6):
            t = singles.tile([128, 1], F32, tag=f"bg_{j}")
            nc.sync.dma_start(t[:], wd["bg"][j * 128 : (j + 1) * 128, :])
            bg_sb.append(t)
        small_vecs = {}
        for name, pn in (
            ("negv", 128),
            ("w0col", 128),
            ("vis_g", 128),
            ("vis_beta", 128),
            ("q1_g", 128),
            ("q1_beta", 128),
            ("q2_b", 8),
        ):
            t = singles.tile([pn, 1], F32, tag=f"sv_{name}")
            nc.sync.dma_start(t[:], wd[name][:])
            small_vecs[name] = t
        eps_sb = singles.tile([128, 1], F32, tag="epsvec")
        nc.vector.memset(eps_sb[:], EPS)

        # ---- pools
        xpool = pool("x0", 2)
        a1pool = pool("a1", 5)
        zpool = pool("z", 8)
        tmpA = pool("tmpA", 9)
        sqpool = pool("sq", 4)
        xspool = pool("xs", 2)
        stspool = pool("sts", 8)
        vopool = pool("vo", 4)       # vision tiles passed A->B
        e96pool = pool("e96", 4)     # enc96 tiles passed A->B
        gpool = pool("gates", 4)
        chpool = pool("ch", 4)       # c + tanh_c f32 temps
        hpool = pool("hh", 4)        # h fp16 tiles passed B->C
        tmpCr = pool("tmpCr", 6)
        qpool = pool("qout", 2)
        psbig = pool("psbig", 3, space="PSUM")
        psstat = pool("psstat", 2, space="PSUM")

        HALVES = (slice(0, TILE), slice(TILE, CH))

        vis_tiles = {}
        enc_tiles = {}
        h_tiles = {}

        # ---------------- phase emitters ----------------
        def emit_A(t):
            cols = slice(t * CH, (t + 1) * CH)
            x0 = xpool.tile([50, CH], F16, tag="x0")
            nc.sync.dma_start(x0[:], obs_t[:, cols])

            a1 = []
            enc96 = e96pool.tile([96, CH], F16, tag="enc96")
            for j, (m0, mn) in enumerate(
                ((0, 128), (128, 128), (256, 128), (384, 16), (400, 96))
            ):
                ps = psbig.tile([mn, CH], F32, tag="ps")
                for h, hs_ in enumerate(HALVES):
                    nc.tensor.matmul(
                        ps[:, hs_], w0_sb[:, m0 : m0 + mn], x0[:, hs_],
                        start=True, stop=True,
                    )
                if j < 3:
                    dst = a1pool.tile([128, CH], F16, tag="a1")
                    if j % 2 == 0:
                        nc.scalar.activation(dst[:], ps[:], AF.Relu, bias=b0_sb[j][:])
                    else:
                        nc.vector.tensor_scalar(
                            dst[:], ps[:], b0_sb[j][:], 0.0, OP.add, OP.max
                        )
                    a1.append(dst)
                elif j == 3:
                    g16 = xpool.tile([16, CH], F16, tag="g16")
                    nc.vector.tensor_scalar(
                        g16[:], ps[:], b0_sb[3][:], 0.0, OP.add, OP.max
                    )
                    a1.append(g16)
                else:
                    nc.scalar.activation(enc96[:], ps[:], AF.Relu, bias=b0_sb[4][:])
            enc_tiles[t] = enc96

            z = []
            for j in range(7):
                m0, mn = j * 128, (128 if j < 6 else 32)
                ps = psbig.tile([mn, CH], F32, tag="ps")
                for h, hs_ in enumerate(HALVES):
                    for kj in range(4):
                        nc.tensor.matmul(
                            ps[:, hs_], k2_sb[kj][:, m0 : m0 + mn], a1[kj][:, hs_],
                            start=(kj == 0), stop=(kj == 3),
                        )
                dst = zpool.tile([mn, CH], F16, tag="z")
                if j % 2 == 0:
                    nc.vector.tensor_scalar(
                        dst[:], ps[:], b2_sb[j][:], 0.0, OP.add, OP.max
                    )
                else:
                    nc.scalar.activation(dst[:], ps[:], AF.Relu, bias=b2_sb[j][:])
                z.append(dst)

            psv = psbig.tile([128, CH], F32, tag="ps")
            for h, hs_ in enumerate(HALVES):
                for kj in range(7):
                    nc.tensor.matmul(
                        psv[:, hs_], vis_sb[kj][:], z[kj][:, hs_],
                        start=(kj == 0), stop=False,
                    )
            # center-in-PSUM LN
            x_s = xspool.tile([128, CH], F16, tag="xs")
            nc.vector.tensor_copy(x_s[:], psv[:])
            for h, hs_ in enumerate(HALVES):
                nc.tensor.matmul(
                    psv[:, hs_], negones128[:], x_s[:, hs_], start=False, stop=True
                )
            xcsq = sqpool.tile([128, CH], F16, tag="sq")
            nc.scalar.activation(xcsq[:], psv[:], AF.Square)
            rr = tmpA.tile([128, CH], F32, tag="tA")
            for h, hs_ in enumerate(HALVES):
                qb = psstat.tile([128, TILE], F32, tag="st")
                nc.tensor.matmul(qb[:], ones128[:], xcsq[:, hs_], start=True, stop=True)
                act_rsqrt(rr[:, hs_], qb[:], eps_sb[:])
            e = tmpA.tile([128, CH], F32, tag="tA")
            nc.vector.tensor_tensor(e[:], psv[:], rr[:], OP.mult)
            vis = vopool.tile([128, CH], F16, tag="visout")
            vact = nc.scalar.activation(
                vis[:], e[:], AF.Relu,
                bias=small_vecs["vis_beta"][:], scale=small_vecs["vis_g"][:],
            )
            vis_tiles[t] = vis
            return vact

        GATE_FN = (AF.Sigmoid, AF.Sigmoid, AF.Tanh, AF.Tanh, AF.Sigmoid, AF.Sigmoid)

        def emit_B(t, act_anchors):
            cols = slice(t * CH, (t + 1) * CH)
            vis = vis_tiles.pop(t)
            enc96 = enc_tiles.pop(t)

            def gate_order(binst):
                # keep the sigmoid-set ACT ops contiguous: schedule them only
                # after this iteration's rsqrt-set ACT work (avoids
                # ACT_TABLE_LOAD thrash between table sets)
                for a in act_anchors:
                    tile.add_dep_helper(
                        binst.ins, a.ins, sync=False, reason="act-set batching"
                    )
                return binst

            gates = []
            for mj in range(6):
                ps = psbig.tile([128, CH], F32, tag="ps")
                for h, hs_ in enumerate(HALVES):
                    nc.tensor.matmul(
                        ps[:, hs_], wih_sb[0][:, mj * 128 : (mj + 1) * 128],
                        vis[:, hs_], start=True, stop=False,
                    )
                    nc.tensor.matmul(
                        ps[:, hs_], wih_sb[1][:, mj * 128 : (mj + 1) * 128],
                        enc96[:, hs_], start=False, stop=True,
                    )
                gt = gpool.tile([128, CH], F32, tag="gate")
                gate_order(
                    nc.scalar.activation(gt[:], ps[:], GATE_FN[mj], bias=bg_sb[mj][:])
                )
                gates.append(gt)
            hts = []
            for j in range(2):
                cj = chpool.tile([128, CH], F32, tag="ch")
                nc.vector.tensor_tensor(cj[:], gates[j][:], gates[2 + j][:], OP.mult)
                nc.sync.dma_start(c_t[j * 128 : (j + 1) * 128, cols], cj[:])
                tcj = chpool.tile([128, CH], F32, tag="ch")
                gate_order(nc.scalar.activation(tcj[:], cj[:], AF.Tanh))
                hj = chpool.tile([128, CH], F32, tag="ch32")
                nc.vector.tensor_tensor(hj[:], gates[4 + j][:], tcj[:], OP.mult)
                nc.sync.dma_start(h_t[j * 128 : (j + 1) * 128, cols], hj[:])
                h16 = hpool.tile([128, CH], F16, tag="hh")
                nc.vector.tensor_copy(h16[:], hj[:])
                hts.append(h16)
            h_tiles[t] = hts

        def emit_C(t):
            cols = slice(t * CH, (t + 1) * CH)
            hs = h_tiles.pop(t)
            hsq = []
            for j in range(2):
                s = sqpool.tile([128, CH], F16, tag="sq")
                nc.scalar.activation(s[:], hs[j][:], AF.Square)
                hsq.append(s)
            r2 = tmpA.tile([128, CH], F32, tag="tA")
            mcp2 = []
            for h, hs_ in enumerate(HALVES):
                mb = psstat.tile([128, TILE], F32, tag="st")
                for kb in range(2):
                    nc.tensor.matmul(
                        mb[:], ones256[:], hs[kb][:, hs_],
                        start=(kb == 0), stop=(kb == 1),
                    )
                mcp = stspool.tile([128, TILE], F32, tag="sts")
                nc.vector.tensor_copy(mcp[:], mb[:])
                mcp2.append(mcp)
                qb = psstat.tile([128, TILE], F32, tag="st")
                for kb in range(2):
                    nc.tensor.matmul(
                        qb[:], ones256[:], hsq[kb][:, hs_],
                        start=(kb == 0), stop=(kb == 1),
                    )
                msq = stspool.tile([128, TILE], F32, tag="sts")
                nc.scalar.activation(msq[:], mcp[:], AF.Square)
                rin = stspool.tile([128, TILE], F32, tag="sts")
                nc.vector.scalar_tensor_tensor(
                    rin[:], qb[:], 1.0, msq[:], OP.mult, OP.subtract
                )
                act_rsqrt(r2[:, hs_], rin[:], eps_sb[:])
            psu = psbig.tile([128, CH], F32, tag="ps")
            for h, hs_ in enumerate(HALVES):
                nc.tensor.matmul(
                    psu[:, hs_], w2t_sb[0][:], hs[0][:, hs_], start=True, stop=False
                )
                nc.tensor.matmul(
                    psu[:, hs_], w2t_sb[1][:], hs[1][:, hs_], start=False, stop=True
                )
            y1 = tmpA.tile([128, CH], F32, tag="tA")
            mr = tmpA.tile([128, CH], F32, tag="tA")
            for h, hs_ in enumerate(HALVES):
                nc.vector.scalar_tensor_tensor(
                    y1[:, hs_], psu[:, hs_], small_vecs["w0col"][:], r2[:, hs_],
                    OP.add, OP.mult,
                )
                nc.vector.tensor_tensor(mr[:, hs_], mcp2[h][:], r2[:, hs_], OP.mult)
            y = tmpCr.tile([128, CH], F16, tag="tC_r")
            nc.vector.scalar_tensor_tensor(
                y[:], mr[:], small_vecs["negv"][:], y1[:], OP.mult, OP.add
            )
            ysq = sqpool.tile([128, CH], F16, tag="sq")
            nc.scalar.activation(ysq[:], y[:], AF.Square)
            r3 = tmpA.tile([128, CH], F32, tag="tA")
            mcp3 = []
            for h, hs_ in enumerate(HALVES):
                mb = psstat.tile([128, TILE], F32, tag="st")
                nc.tensor.matmul(mb[:], ones128[:], y[:, hs_], start=True, stop=True)
                mcp = stspool.tile([128, TILE], F32, tag="sts")
                nc.vector.tensor_copy(mcp[:], mb[:])
                mcp3.append(mcp)
                qb = psstat.tile([128, TILE], F32, tag="st")
                nc.tensor.matmul(qb[:], ones128[:], ysq[:, hs_], start=True, stop=True)
                msq = stspool.tile([128, TILE], F32, tag="sts")
                nc.scalar.activation(msq[:], mcp[:], AF.Square)
                rin = stspool.tile([128, TILE], F32, tag="sts")
                nc.vector.scalar_tensor_tensor(
                    rin[:], qb[:], 1.0, msq[:], OP.mult, OP.subtract
                )
                act_rsqrt(r3[:, hs_], rin[:], eps_sb[:])
            d3 = tmpA.tile([128, CH], F32, tag="tA")
            for h, hs_ in enumerate(HALVES):
                nc.vector.scalar_tensor_tensor(
                    d3[:, hs_], mcp3[h][:], -1.0, y[:, hs_], OP.mult, OP.add
                )
            e3 = tmpA.tile([128, CH], F32, tag="tA")
            nc.vector.tensor_tensor(e3[:], d3[:], r3[:], OP.mult)
            hq = tmpCr.tile([128, CH], F16, tag="tC_r")
            hq_act = nc.scalar.activation(
                hq[:], e3[:], AF.Relu,
                bias=small_vecs["q1_beta"][:], scale=small_vecs["q1_g"][:],
            )
            qsb = qpool.tile([NACT, CH], F32, tag="qsb")
            for h, hs_ in enumerate(HALVES):
                psq = psstat.tile([NACT, TILE], F32, tag="st")
                nc.tensor.matmul(psq[:], q2t_sb[:], hq[:, hs_], start=True, stop=True)
                nc.scalar.activation(
                    qsb[:, hs_], psq[:], AF.Identity, bias=small_vecs["q2_b"][:]
                )
            nc.sync.dma_start(q_t[:, cols], qsb[:])
            return hq_act

        # ---- software-pipelined phase schedule: A(t) | C(t-2) | B(t-1)
        # (A and C share the rsqrt ACT table set; B's sigmoid set goes last
        # in each iteration so the ScalarE switches sets only twice per iter)
        for it in range(N_CHUNKS + 2):
            anchors = []
            if it < N_CHUNKS:
                anchors.append(emit_A(it))
            if it >= 2:
                anchors.append(emit_C(it - 2))
            if 1 <= it <= N_CHUNKS:
                emit_B(it - 1, anchors)

        for p in reversed(ctx_pools):
            p.__exit__(None, None, None)

    _split_multi_waits(nc, mybir)
    return nc


def _get_nc():
    if "nc" not in _CACHE:
        _CACHE["nc"] = _build_nc()
    return _CACHE["nc"]


# --------------------------------------------------------------------------
# numpy fallback (general h0/c0 path; bit-faithful to the reference math)
# --------------------------------------------------------------------------
def _numpy_reference(inp):
    p = {k: np.asarray(v, np.float32) for k, v in inp.items()}
    obs, h0, c0 = p["obs"], p["h0"], p["c0"]
    Bx = obs.shape[0]

    def ln(x, g, b):
        m = x.mean(-1, keepdims=True)
        v = ((x - m) ** 2).mean(-1, keepdims=True)
        return (x - m) / np.sqrt(v + EPS) * g + b

    grid = obs[:, :25].reshape(Bx, 1, 5, 5)
    pos, met, aff = obs[:, 25:27], obs[:, 27:35], obs[:, 35:50]

    def conv(x, w, b):
        Co, Ci = w.shape[0], w.shape[1]
        xp = np.zeros((Bx, Ci, 7, 7), np.float32)
        xp[:, :, 1:6, 1:6] = x
        out = np.zeros((Bx, Co, 5, 5), np.float32)
        for kh in range(3):
            for kw in range(3):
                patch = xp[:, :, kh : kh + 5, kw : kw + 5]
                out += np.einsum("bchw,oc->bohw", patch, w[:, :, kh, kw])
        return out + b[None, :, None, None]

    x = np.maximum(conv(grid, p["conv1_w"], p["conv1_b"]), 0)
    x = np.maximum(conv(x, p["conv2_w"], p["conv2_b"]), 0)
    x = x.reshape(Bx, -1)
    vision = np.maximum(
        ln(x @ p["vis_w"].T + p["vis_b"], p["vis_g"], p["vis_beta"]), 0
    )
    pf = np.maximum(pos @ p["pos_w"].T + p["pos_b"], 0)
    mf = np.maximum(met @ p["met_w"].T + p["met_b"], 0)
    af = np.maximum(aff @ p["aff_w"].T + p["aff_b"], 0)
    comb = np.concatenate([vision, pf, mf, af], axis=1)
    gates = comb @ p["w_ih"].T + p["b_ih"] + h0 @ p["w_hh"].T + p["b_hh"]
    i, f, g, o = np.split(gates, 4, axis=1)
    sig = lambda v: 1.0 / (1.0 + np.exp(-v))
    i, f, g, o = sig(i), sig(f), np.tanh(g), sig(o)
    c_new = f * c0 + i * g
    h_new = o * np.tanh(c_new)
    lstm_out = ln(h_new, p["ln_g"], p["ln_b"])
    hq = np.maximum(
        ln(lstm_out @ p["q1_w"].T + p["q1_b"], p["q1_g"], p["q1_beta"]), 0
    )
    qv = hq @ p["q2_w"].T + p["q2_b"]
    return qv.astype(np.float32), h_new.astype(np.float32), c_new.astype(np.float32)


# --------------------------------------------------------------------------
# entry point
# --------------------------------------------------------------------------
MM_PARAMS = {"w0", "k2", "vist", "wiht", "w2t", "q2t", "ones128", "ones256", "negones128"}


def run_sharded(inputs, trace=False):
    """Build per-core input maps, run the SPMD kernel, return (results, bench)."""
    from concourse.bass_utils import run_bass_kernel_spmd

    w = _prep_weights(inputs)
    for k in MM_PARAMS:
        w[k] = np.ascontiguousarray(w[k].astype(np.float16))
    obs = np.asarray(inputs["obs"], np.float32)
    obs_t = np.ascontiguousarray(obs.T.astype(np.float16))  # [50, B] fp16

    in_maps = []
    for ci in range(N_CORES):
        m = {k: w[k] for k in WEIGHT_SHAPES}
        m["obs_t"] = np.ascontiguousarray(obs_t[:, ci * B_CORE : (ci + 1) * B_CORE])
        in_maps.append(m)

    nc = _get_nc()
    res = run_bass_kernel_spmd(
        nc, in_maps, list(range(N_CORES)), trace=trace
    )
    return res


def kernel(**inputs):
    h0 = np.asarray(inputs["h0"])
    c0 = np.asarray(inputs["c0"])
    if np.any(h0) or np.any(c0):
        return _numpy_reference(inputs)

    res = run_sharded(inputs, trace=False)

    q = np.empty((B, NACT), np.float32)
    h = np.empty((B, HID), np.float32)
    c = np.empty((B, HID), np.float32)
    for ci in range(N_CORES):
        sl = slice(ci * B_CORE, (ci + 1) * B_CORE)
        out = res.results[ci]
        q[sl] = out["q_t"].T
        h[sl] = out["h_t"].T
        c[sl] = out["c_t"].T
    return q, h, c
